# revision 1
# baseline (speedup 1.0000x reference)
"""Jamba sparse-MoE block on 8 Trainium2 NeuronCores (expert-parallel).

Strategy
--------
- Routing (router matmul + softmax + top-2) is computed with jax on the host
  using the exact op sequence of the reference so expert selection matches
  bit-for-bit (one token in the dataset has a top2/top3 probability gap of
  ~5e-7; any rounding difference there would flip its expert assignment).
- Tokens are dispatched (gathered) per expert on the host; core e runs the
  dense gate/up/silu/mul/down FFN of expert e over its ~2k assigned tokens.
  This is the "all-to-all dispatch by top_k_index + expert-parallel weights"
  sharding, with the dispatch done at input-sharding time.
- Each core's Bass kernel is PE-bound and runs matmuls as float32r (full PE
  rate at N>=256, ~1.5e-4 relative rounding) with fp32 PSUM accumulation:
    phase A: hid = silu(x @ gw.T) * (x @ uw.T)   [F x C], staged to DRAM
    phase B: y = (w_token * (hid.T @ dw.T))      [C x H]
  The down-projection weights are fully cached in SBUF (one half preloaded
  during phase A, the other after the x tile is freed), so hid streams
  through phase B exactly once and the PE stays the bottleneck.
- Outputs are scatter-added back into the full [T, H] buffer on the host
  (each token appears in exactly two experts' outputs).
"""

import math
import numpy as np
from contextlib import ExitStack

B, S, H, F, E, TOP_K = 4, 2048, 1024, 4096, 8, 2
T = B * S
N_CORES = 8
P = 128
HC = H // P  # 8 h-chunks
FB = F // P  # 32 f-blocks


def _token_tiles(C):
    assert C % 256 == 0 and C >= 256
    tiles = [512] * (C // 512)
    if C % 512:
        tiles.append(C % 512)
    return tiles


_PROGRAM_CACHE = {}


def _build_program(C, H_=H, F_=F, act="Silu"):
    """SPMD program for one expert's FFN over C token slots."""
    key = (C, H_, F_, act)
    if key in _PROGRAM_CACHE:
        return _PROGRAM_CACHE[key]
    import concourse.bacc as bacc
    import concourse.mybir as mybir
    import concourse.tile as tile

    HC = H_ // P
    FB = F_ // P
    FBH = (3 * FB) // 4  # dw cache split: big half preloaded during phase A
    NH = max(1, H_ // 512)  # matmul slices along H
    HW_ = H_ // NH
    f32 = mybir.dt.float32
    f32r = mybir.dt.float32r
    AF = mybir.ActivationFunctionType
    NT128 = C // P
    NT256 = C // 256
    tiles = _token_tiles(C)

    nc = bacc.Bacc("TRN2", target_bir_lowering=False, debug=False, num_devices=N_CORES)

    x_d = nc.dram_tensor("x", [P, HC, C], f32r, kind="ExternalInput")
    gw_d = nc.dram_tensor("gw", [FB, P, HC, P], f32r, kind="ExternalInput")
    uw_d = nc.dram_tensor("uw", [FB, P, HC, P], f32r, kind="ExternalInput")
    dw_d = nc.dram_tensor("dw", [P, FB, H_], f32r, kind="ExternalInput")
    wt_d = nc.dram_tensor("wt", [NT128, P], f32, kind="ExternalInput")
    y_d = nc.dram_tensor("y", [NT128, P, H_], f32, kind="ExternalOutput")
    hid_d = nc.dram_tensor("hid", [FB, P, C], f32r)  # internal staging

    with tile.TileContext(nc) as tc:
        with ExitStack() as ctx:
            wtpool = ctx.enter_context(tc.tile_pool(name="wtp", bufs=1))
            dlpool = ctx.enter_context(tc.tile_pool(name="dlp", bufs=1))

            wt_t = wtpool.tile([P, NT128], f32)
            nc.sync.dma_start(wt_t[:], wt_d.ap().rearrange("n p -> p n"))
            # first dw half: preloaded while phase A runs
            dw_lo = dlpool.tile([P, FBH, H_], f32r)
            nc.gpsimd.dma_start(dw_lo[:], dw_d.ap()[:, :FBH, :])

            # ---- Phase A: hid[f, t] = silu(g) * u, staged to DRAM ----
            with ExitStack() as actx:
                psa = actx.enter_context(tc.tile_pool(name="psa", bufs=3, space="PSUM"))
                xpool = actx.enter_context(tc.tile_pool(name="xp", bufs=1))
                gwpool = actx.enter_context(tc.tile_pool(name="gwp", bufs=3))
                uwpool = actx.enter_context(tc.tile_pool(name="uwp", bufs=3))
                sgpool = actx.enter_context(tc.tile_pool(name="sgp", bufs=2))
                hspool = actx.enter_context(tc.tile_pool(name="hsp", bufs=3))

                x_t = xpool.tile([P, HC, C], f32r)
                t0 = 0
                xchunks = [256, 256] + list(tiles[1:]) if tiles[0] == 512 else tiles
                for nt in xchunks:
                    nc.sync.dma_start(
                        x_t[:, :, t0 : t0 + nt], x_d.ap()[:, :, t0 : t0 + nt]
                    )
                    t0 += nt

                for fb in range(FB):
                    gw_t = gwpool.tile([P, HC, P], f32r)
                    nc.sync.dma_start(gw_t[:], gw_d.ap()[fb])
                    uw_t = uwpool.tile([P, HC, P], f32r)
                    nc.sync.dma_start(uw_t[:], uw_d.ap()[fb])
                    t0 = 0
                    for nt in tiles:
                        ps_g = psa.tile([P, 512], f32, name="ps_g")[:, :nt]
                        ps_u = psa.tile([P, 512], f32, name="ps_u")[:, :nt]
                        for hc in range(HC):
                            nc.tensor.matmul(
                                ps_g,
                                gw_t[:, hc, :],
                                x_t[:, hc, t0 : t0 + nt],
                                start=(hc == 0),
                                stop=(hc == HC - 1),
                            )
                        for hc in range(HC):
                            nc.tensor.matmul(
                                ps_u,
                                uw_t[:, hc, :],
                                x_t[:, hc, t0 : t0 + nt],
                                start=(hc == 0),
                                stop=(hc == HC - 1),
                            )
                        sg = sgpool.tile([P, 512], f32, name="sg")[:, :nt]
                        nc.scalar.activation(sg, ps_g, getattr(AF, act))
                        hid_sb = hspool.tile([P, 512], f32r, name="hid_sb")[:, :nt]
                        nc.vector.tensor_mul(hid_sb, sg, ps_u)
                        nc.scalar.dma_start(hid_d.ap()[fb][:, t0 : t0 + nt], hid_sb)
                        t0 += nt

            # ---- Phase B: y[t, :] = w[t] * (hid[:, t].T @ dw.T) ----
            psb = ctx.enter_context(tc.tile_pool(name="psb", bufs=3, space="PSUM"))
            dhpool = ctx.enter_context(tc.tile_pool(name="dhp", bufs=1))
            htpool = ctx.enter_context(tc.tile_pool(name="htp", bufs=2))
            ypool = ctx.enter_context(tc.tile_pool(name="yp", bufs=2))

            dw_hi = dhpool.tile([P, FB - FBH, H_], f32r)
            nc.gpsimd.dma_start(dw_hi[:], dw_d.ap()[:, FBH:, :])

            for tt2 in range(NT256):
                hid_t = htpool.tile([P, FB, 256], f32r, name="hid_t")
                nc.sync.dma_start(
                    hid_t[:], hid_d.ap()[:, :, tt2 * 256 : (tt2 + 1) * 256]
                    .rearrange("b f t -> f b t")
                )
                for sub in range(2):
                    tt = tt2 * 2 + sub
                    ps_y = psb.tile([P, H_], f32, name="ps_y")
                    for fb in range(FB):
                        dwt = dw_lo if fb < FBH else dw_hi
                        fbi = fb if fb < FBH else fb - FBH
                        for nh in range(NH):
                            nc.tensor.matmul(
                                ps_y[:, nh * HW_ : (nh + 1) * HW_],
                                hid_t[:, fb, sub * P : (sub + 1) * P],
                                dwt[:, fbi, nh * HW_ : (nh + 1) * HW_],
                                start=(fb == 0),
                                stop=(fb == FB - 1),
                            )
                    y_sb = ypool.tile([P, H_], f32, name="y_sb")
                    nc.scalar.activation(
                        y_sb[:], ps_y[:], AF.Copy, scale=wt_t[:, tt : tt + 1]
                    )
                    nc.scalar.dma_start(y_d.ap()[tt], y_sb[:])
    nc.compile()
    _PROGRAM_CACHE[key] = nc
    return nc


def _routing(hidden_states, router_w):
    """Replicate the reference's routing ops exactly (same jax ops, default
    platform) so top-2 selection matches bit-for-bit."""
    import jax
    import jax.numpy as jnp

    x = jnp.asarray(hidden_states).reshape(-1, H)
    router_logits = x @ jnp.asarray(router_w).T
    routing_weights = jax.nn.softmax(router_logits.astype(jnp.float32), axis=-1)
    top_k_weights, top_k_index = jax.lax.top_k(routing_weights, TOP_K)
    return np.asarray(top_k_index), np.asarray(top_k_weights, dtype=np.float32)


def kernel(hidden_states, router_w, gate_w, up_w, down_w):
    from concourse.bass_utils import run_bass_kernel_spmd

    hidden_states = np.asarray(hidden_states, dtype=np.float32)
    router_w = np.asarray(router_w, dtype=np.float32)
    gate_w = np.asarray(gate_w, dtype=np.float32)
    up_w = np.asarray(up_w, dtype=np.float32)
    down_w = np.asarray(down_w, dtype=np.float32)

    tki, tkw = _routing(hidden_states, router_w)
    xf = hidden_states.reshape(T, H)

    idx_list, w_list = [], []
    for e in range(E):
        sel = tki == e  # [T, 2]
        tok = sel.any(axis=1)
        idx = np.nonzero(tok)[0]
        w = np.where(sel[:, 0], tkw[:, 0], tkw[:, 1])[idx]
        idx_list.append(idx)
        w_list.append(w.astype(np.float32))

    max_ne = max(len(i) for i in idx_list)
    C = max(512, int(math.ceil(max_ne / 256.0)) * 256)
    NT128 = C // P

    nc = _build_program(C)

    in_maps = []
    for e in range(E):
        idx, w = idx_list[e], w_list[e]
        ne = len(idx)
        xg = np.zeros((C, H), np.float32)
        xg[:ne] = xf[idx]
        wp = np.zeros((C,), np.float32)
        wp[:ne] = w
        in_maps.append(
            {
                "x": np.ascontiguousarray(
                    xg.T.reshape(HC, P, C).transpose(1, 0, 2)
                ),
                "gw": np.ascontiguousarray(
                    gate_w[e].reshape(FB, P, HC, P).transpose(0, 3, 2, 1)
                ),
                "uw": np.ascontiguousarray(
                    up_w[e].reshape(FB, P, HC, P).transpose(0, 3, 2, 1)
                ),
                "dw": np.ascontiguousarray(
                    down_w[e].T.reshape(FB, P, H).transpose(1, 0, 2)
                ),
                "wt": np.ascontiguousarray(wp.reshape(NT128, P)),
            }
        )

    res = run_bass_kernel_spmd(nc, in_maps, core_ids=list(range(N_CORES)))

    out = np.zeros((T, H), np.float32)
    for e in range(E):
        idx = idx_list[e]
        y = res.results[e]["y"].reshape(C, H)
        out[idx] += y[: len(idx)]
    return out.reshape(B, S, H)



# revision 3
# speedup vs baseline: 1.1541x; 1.1541x over previous
"""Jamba sparse-MoE block on 8 Trainium2 NeuronCores (expert-parallel).

Strategy
--------
- Routing (router matmul + softmax + top-2) is computed with jax on the host
  using the exact op sequence of the reference so expert selection matches
  bit-for-bit (one token in the dataset has a top2/top3 probability gap of
  ~5e-7; any rounding difference there would flip its expert assignment).
- Tokens are dispatched (gathered) per expert on the host; core e runs the
  dense gate/up/silu/mul/down FFN of expert e over its ~2k assigned tokens.
- All matmul operands are bf16 (same PE rate as float32r on TRN2 — 1 row/cyc
  — but half the DMA bytes and no >=256 free-dim constraint, so capacity can
  be padded to 128 instead of 256). PSUM accumulation stays fp32.
- The token range is processed in groups of 1024. Per group, phase A
  (hid = silu(x@gw.T) * (x@uw.T)) keeps hid in SBUF as bf16 — no DRAM
  round-trip — and phase B (y = wt * (hid.T @ dw.T)) immediately consumes it.
  gate/up weights are re-streamed per group (~17MB/group, fully hidden under
  ~380us of PE work per group); down weights stay resident in SBUF.
- Outputs are scatter-added back into the full [T, H] buffer on the host
  (each token appears in exactly two experts' outputs).
"""

import math
import numpy as np
from contextlib import ExitStack

B, S, H, F, E, TOP_K = 4, 2048, 1024, 4096, 8, 2
T = B * S
N_CORES = 8
P = 128
HC = H // P  # 8 h-chunks
FB = F // P  # 32 f-blocks
GSZ = 1024  # tokens per phase-A/phase-B group


def _token_tiles(g):
    """512-token phase-A tiles covering a group of g tokens (g % 128 == 0)."""
    tiles = [512] * (g // 512)
    if g % 512:
        tiles.append(g % 512)
    return tiles


def _groups(C):
    out, t0 = [], 0
    while t0 < C:
        g = min(GSZ, C - t0)
        out.append((t0, g))
        t0 += g
    return out


_PROGRAM_CACHE = {}


def _build_program(C, H_=H, F_=F, act="Silu"):
    """SPMD program for one expert's FFN over C token slots (C % 128 == 0)."""
    key = (C, H_, F_, act)
    if key in _PROGRAM_CACHE:
        return _PROGRAM_CACHE[key]
    import concourse.bacc as bacc
    import concourse.mybir as mybir
    import concourse.tile as tile

    HC = H_ // P
    FB = F_ // P
    f32 = mybir.dt.float32
    bf16 = mybir.dt.bfloat16
    AF = mybir.ActivationFunctionType
    NT128 = C // P

    nc = bacc.Bacc("TRN2", target_bir_lowering=False, debug=False, num_devices=N_CORES)

    x_d = nc.dram_tensor("x", [P, HC, C], bf16, kind="ExternalInput")
    gw_d = nc.dram_tensor("gw", [FB, P, HC, P], bf16, kind="ExternalInput")
    uw_d = nc.dram_tensor("uw", [FB, P, HC, P], bf16, kind="ExternalInput")
    dw_d = nc.dram_tensor("dw", [P, FB, H_], bf16, kind="ExternalInput")
    wt_d = nc.dram_tensor("wt", [NT128, P], f32, kind="ExternalInput")
    y_d = nc.dram_tensor("y", [NT128, P, H_], f32, kind="ExternalOutput")

    with tile.TileContext(nc) as tc:
        with ExitStack() as ctx:
            wtpool = ctx.enter_context(tc.tile_pool(name="wtp", bufs=1))
            xpool = ctx.enter_context(tc.tile_pool(name="xp", bufs=1))
            dwpool = ctx.enter_context(tc.tile_pool(name="dwp", bufs=1))
            gwpool = ctx.enter_context(tc.tile_pool(name="gwp", bufs=3))
            uwpool = ctx.enter_context(tc.tile_pool(name="uwp", bufs=3))
            sgpool = ctx.enter_context(tc.tile_pool(name="sgp", bufs=2))
            hidpool = ctx.enter_context(tc.tile_pool(name="hidp", bufs=1))
            ypool = ctx.enter_context(tc.tile_pool(name="yp", bufs=2))
            psg = ctx.enter_context(tc.tile_pool(name="psg", bufs=2, space="PSUM"))
            psu = ctx.enter_context(tc.tile_pool(name="psu", bufs=2, space="PSUM"))
            psy = ctx.enter_context(tc.tile_pool(name="psy", bufs=2, space="PSUM"))

            # x streams on the gpsimd queue so it doesn't delay gate/up
            # weights on the sync queue; the first 512-token chunk goes
            # per-h-chunk so the very first matmul only waits on a 128KB
            # transfer.
            x_t = xpool.tile([P, HC, C], bf16)
            for hc in range(HC):
                nc.gpsimd.dma_start(x_t[:, hc, 0:512], x_d.ap()[:, hc, 0:512])
            t0 = 512
            while t0 < C:
                nt = min(512, C - t0)
                nc.gpsimd.dma_start(x_t[:, :, t0 : t0 + nt], x_d.ap()[:, :, t0 : t0 + nt])
                t0 += nt

            wt_t = wtpool.tile([P, NT128], f32)
            nc.sync.dma_start(wt_t[:], wt_d.ap().rearrange("n p -> p n"))

            # down weights: fully resident, streamed on the gpsimd queue
            # (first needed ~300us in, at the first group's phase B)
            dw_t = dwpool.tile([P, FB, H_], bf16)
            nc.gpsimd.dma_start(dw_t[:, : FB // 2, :], dw_d.ap()[:, : FB // 2, :])
            nc.gpsimd.dma_start(dw_t[:, FB // 2 :, :], dw_d.ap()[:, FB // 2 :, :])

            for t0, g in _groups(C):
                # ---- Phase A: hid[f, t] = silu(g) * u, bf16 in SBUF ----
                hid_t = hidpool.tile([P, FB, GSZ], bf16, name="hid_t")
                for fb in range(FB):
                    gw_t = gwpool.tile([P, HC, P], bf16, name="gw_t")
                    nc.sync.dma_start(gw_t[:], gw_d.ap()[fb])
                    uw_t = uwpool.tile([P, HC, P], bf16, name="uw_t")
                    nc.sync.dma_start(uw_t[:], uw_d.ap()[fb])
                    tt = 0
                    for nt in _token_tiles(g):
                        ps_g = psg.tile([P, 512], f32, name="ps_g")[:, :nt]
                        ps_u = psu.tile([P, 512], f32, name="ps_u")[:, :nt]
                        for hc in range(HC):
                            nc.tensor.matmul(
                                ps_g,
                                gw_t[:, hc, :],
                                x_t[:, hc, t0 + tt : t0 + tt + nt],
                                start=(hc == 0),
                                stop=(hc == HC - 1),
                            )
                        for hc in range(HC):
                            nc.tensor.matmul(
                                ps_u,
                                uw_t[:, hc, :],
                                x_t[:, hc, t0 + tt : t0 + tt + nt],
                                start=(hc == 0),
                                stop=(hc == HC - 1),
                            )
                        sg = sgpool.tile([P, 512], f32, name="sg")[:, :nt]
                        nc.scalar.activation(sg, ps_g, getattr(AF, act))
                        nc.vector.tensor_mul(hid_t[:, fb, tt : tt + nt], sg, ps_u)
                        tt += nt

                # ---- Phase B: y[t, :] = wt[t] * (hid[:, t].T @ dw.T) ----
                for sub in range(g // P):
                    tt128 = t0 // P + sub
                    ps_y = psy.tile([P, H_], f32, name="ps_y")
                    for fb in range(FB):
                        for hh in range(2):
                            nc.tensor.matmul(
                                ps_y[:, hh * 512 : (hh + 1) * 512],
                                hid_t[:, fb, sub * P : (sub + 1) * P],
                                dw_t[:, fb, hh * 512 : (hh + 1) * 512],
                                start=(fb == 0),
                                stop=(fb == FB - 1),
                            )
                    y_sb = ypool.tile([P, H_], f32, name="y_sb")
                    nc.scalar.activation(
                        y_sb[:], ps_y[:], AF.Copy, scale=wt_t[:, tt128 : tt128 + 1]
                    )
                    nc.scalar.dma_start(y_d.ap()[tt128], y_sb[:])
    nc.compile()
    _PROGRAM_CACHE[key] = nc
    return nc


def _routing(hidden_states, router_w):
    """Replicate the reference's routing ops exactly (same jax ops, default
    platform) so top-2 selection matches bit-for-bit."""
    import jax
    import jax.numpy as jnp

    x = jnp.asarray(hidden_states).reshape(-1, H)
    router_logits = x @ jnp.asarray(router_w).T
    routing_weights = jax.nn.softmax(router_logits.astype(jnp.float32), axis=-1)
    top_k_weights, top_k_index = jax.lax.top_k(routing_weights, TOP_K)
    return np.asarray(top_k_index), np.asarray(top_k_weights, dtype=np.float32)


def kernel(hidden_states, router_w, gate_w, up_w, down_w):
    import ml_dtypes
    from concourse.bass_utils import run_bass_kernel_spmd

    bf16 = ml_dtypes.bfloat16
    hidden_states = np.asarray(hidden_states, dtype=np.float32)
    router_w = np.asarray(router_w, dtype=np.float32)
    gate_w = np.asarray(gate_w, dtype=np.float32)
    up_w = np.asarray(up_w, dtype=np.float32)
    down_w = np.asarray(down_w, dtype=np.float32)

    tki, tkw = _routing(hidden_states, router_w)
    xf = hidden_states.reshape(T, H).astype(bf16)

    idx_list, w_list = [], []
    for e in range(E):
        sel = tki == e  # [T, 2]
        tok = sel.any(axis=1)
        idx = np.nonzero(tok)[0]
        w = np.where(sel[:, 0], tkw[:, 0], tkw[:, 1])[idx]
        idx_list.append(idx)
        w_list.append(w.astype(np.float32))

    max_ne = max(len(i) for i in idx_list)
    C = max(256, int(math.ceil(max_ne / 128.0)) * 128)
    NT128 = C // P

    nc = _build_program(C)

    in_maps = []
    for e in range(E):
        idx, w = idx_list[e], w_list[e]
        ne = len(idx)
        xg = np.zeros((C, H), bf16)
        xg[:ne] = xf[idx]
        wp = np.zeros((C,), np.float32)
        wp[:ne] = w
        in_maps.append(
            {
                "x": np.ascontiguousarray(
                    xg.T.reshape(HC, P, C).transpose(1, 0, 2)
                ),
                "gw": np.ascontiguousarray(
                    gate_w[e].astype(bf16).reshape(FB, P, HC, P).transpose(0, 3, 2, 1)
                ),
                "uw": np.ascontiguousarray(
                    up_w[e].astype(bf16).reshape(FB, P, HC, P).transpose(0, 3, 2, 1)
                ),
                "dw": np.ascontiguousarray(
                    down_w[e].T.astype(bf16).reshape(FB, P, H).transpose(1, 0, 2)
                ),
                "wt": np.ascontiguousarray(wp.reshape(NT128, P)),
            }
        )

    res = run_bass_kernel_spmd(nc, in_maps, core_ids=list(range(N_CORES)))

    out = np.zeros((T, H), np.float32)
    for e in range(E):
        idx = idx_list[e]
        y = res.results[e]["y"].reshape(C, H)
        out[idx] += y[: len(idx)]
    return out.reshape(B, S, H)


# revision 15
# speedup vs baseline: 1.2495x; 1.0827x over previous
"""Jamba sparse-MoE block on 8 Trainium2 NeuronCores.

Strategy: expert-parallel with tensor-parallel ffn halves, host dispatch
--------
- Routing (router matmul + softmax + top-2) is computed with jax on the host
  using the exact op sequence of the reference so expert selection matches
  bit-for-bit (one token in the dataset has a top2/top3 probability gap of
  ~5e-7; any rounding difference there would flip its expert assignment).
- Every expert's FFN dim is split in half (F=4096 -> 2x2048). Each core runs
  two segments: the F-half of a heavy expert (by token load) paired with the
  F-half of a light expert, so per-core work is balanced to ~(C1+C2)/2
  tokens instead of being pinned by the heaviest expert. The two partial
  outputs per (token, expert) are summed on the host during scatter-add.
- All matmul operands are bf16 (same PE rate as float32r on TRN2 — 1 row/cyc
  — but half the DMA bytes and no >=256 free-dim constraint). PSUM fp32.
- Each segment's token range is processed in groups of ~1k tokens. Per
  group, phase A (hid = silu(x@gw.T) * (x@uw.T)) keeps hid in SBUF as bf16 —
  no DRAM round-trip — and phase B (y = wt * (hid.T @ dw.T)) immediately
  consumes it. gate/up weights are re-streamed per group; down weights stay
  resident in SBUF.
- The cost model serializes all DMA on one device, so every input load is
  emitted on the sync queue in consumption-deadline order; y stores use the
  scalar queue.
"""

import math
import numpy as np
from contextlib import ExitStack

B, S, H, F, E, TOP_K = 4, 2048, 1024, 4096, 8, 2
T = B * S
N_CORES = 8
P = 128
HC = H // P  # 8 h-chunks
FB = F // P  # 32 f-blocks total; 16 per F-half segment
FBH = FB // 2
F2 = F // 2
GSZ = 1024  # target tokens per phase-A/phase-B group


def _token_tiles(g):
    """512-token phase-A tiles covering a group of g tokens (g % 128 == 0)."""
    tiles = [512] * (g // 512)
    if g % 512:
        tiles.append(g % 512)
    return tiles


def _split_groups(Cs):
    """Split a segment of Cs tokens into near-equal 128-multiple groups of at
    most GSZ+128: equal groups keep per-fb PE work far above the per-fb
    weight DMA time, so the gate/up weight stream never starves the PE."""
    nt = Cs // P
    n = max(2, -(-nt // (GSZ // P + 1)))
    out, t0 = [], 0
    for i in range(n):
        take = (nt * (i + 1) // n - nt * i // n) * P
        if take:
            out.append((t0, take))
            t0 += take
    return out


_PROGRAM_CACHE = {}


def _build_program(C1, C2, H_=H, act="Silu"):
    """SPMD program: two expert-half segments of C1 and C2 token slots."""
    key = ((C1, C2), H_, F, act)
    if key in _PROGRAM_CACHE:
        return _PROGRAM_CACHE[key]
    import concourse.bacc as bacc
    import concourse.mybir as mybir
    import concourse.tile as tile

    HC = H_ // P
    f32 = mybir.dt.float32
    bf16 = mybir.dt.bfloat16
    AF = mybir.ActivationFunctionType
    C = C1 + C2
    NT128 = C // P

    # (token_offset, group_len, fb_lo) — fb blocks 0..16 are segment 1's
    # F-half, 16..32 segment 2's.
    groups = [(t0, g, 0) for t0, g in _split_groups(C1)] + [
        (C1 + t0, g, FBH) for t0, g in _split_groups(C2)
    ]

    nc = bacc.Bacc("TRN2", target_bir_lowering=False, debug=False, num_devices=N_CORES)

    x_d = nc.dram_tensor("x", [P, HC, C], bf16, kind="ExternalInput")
    gw_d = nc.dram_tensor("gw", [FB, P, HC, P], bf16, kind="ExternalInput")
    uw_d = nc.dram_tensor("uw", [FB, P, HC, P], bf16, kind="ExternalInput")
    dw_d = nc.dram_tensor("dw", [P, FB, H_], bf16, kind="ExternalInput")
    wt_d = nc.dram_tensor("wt", [NT128, P], f32, kind="ExternalInput")
    y_d = nc.dram_tensor("y", [NT128, P, H_], f32, kind="ExternalOutput")

    with tile.TileContext(nc) as tc:
        with ExitStack() as ctx:
            wtpool = ctx.enter_context(tc.tile_pool(name="wtp", bufs=1))
            xpool = ctx.enter_context(tc.tile_pool(name="xp", bufs=1))
            dwpool = ctx.enter_context(tc.tile_pool(name="dwp", bufs=1))
            gwpool = ctx.enter_context(tc.tile_pool(name="gwp", bufs=4))
            uwpool = ctx.enter_context(tc.tile_pool(name="uwp", bufs=4))
            sgpool = ctx.enter_context(tc.tile_pool(name="sgp", bufs=2))
            hidpool = ctx.enter_context(tc.tile_pool(name="hidp", bufs=1))
            ypool = ctx.enter_context(tc.tile_pool(name="yp", bufs=4))
            psg = ctx.enter_context(tc.tile_pool(name="psg", bufs=2, space="PSUM"))
            psu = ctx.enter_context(tc.tile_pool(name="psu", bufs=2, space="PSUM"))
            psy = ctx.enter_context(tc.tile_pool(name="psy", bufs=4, space="PSUM"))

            x_t = xpool.tile([P, HC, C], bf16)
            dw_t = dwpool.tile([P, FB, H_], bf16)
            wt_t = wtpool.tile([P, NT128], f32)

            x_chunks = []
            t0 = 0
            while t0 < C:
                nt = min(512, C - t0)
                x_chunks.append((t0, nt))
                t0 += nt

            def load_x(chunk, per_hc):
                xt0, xnt = chunk
                if per_hc:
                    for hc in range(HC):
                        nc.sync.dma_start(
                            x_t[:, hc, xt0 : xt0 + xnt], x_d.ap()[:, hc, xt0 : xt0 + xnt]
                        )
                else:
                    nc.sync.dma_start(
                        x_t[:, :, xt0 : xt0 + xnt], x_d.ap()[:, :, xt0 : xt0 + xnt]
                    )

            hid_max = max(g for _, g, _ in groups)
            for gi, (t0, g, fb_lo) in enumerate(groups):
                # ---- Phase A: hid[f, t] = silu(g) * u, bf16 in SBUF ----
                hid_t = hidpool.tile([P, FBH, hid_max], bf16, name="hid_t")
                for fbi in range(FBH):
                    fb = fb_lo + fbi
                    gw_t = gwpool.tile([P, HC, P], bf16, name="gw_t")
                    nc.sync.dma_start(gw_t[:], gw_d.ap()[fb])
                    if gi == 0 and fbi == 0:
                        load_x(x_chunks.pop(0), per_hc=True)
                    uw_t = uwpool.tile([P, HC, P], bf16, name="uw_t")
                    nc.sync.dma_start(uw_t[:], uw_d.ap()[fb])
                    if gi == 0 and fbi == 0 and x_chunks:
                        load_x(x_chunks.pop(0), per_hc=True)
                    if gi == 0 and fbi in (1, 2, 3) and x_chunks:
                        load_x(x_chunks.pop(0), per_hc=False)
                    if gi == 0 and fbi == 4:
                        while x_chunks:
                            load_x(x_chunks.pop(0), per_hc=False)
                        nc.sync.dma_start(wt_t[:], wt_d.ap().rearrange("n p -> p n"))
                    # down-weight quarters, in deadline order (first segment's
                    # phase B first)
                    if gi == 0 and fbi in (6, 10):
                        q = FB // 4 * ((fbi - 6) // 4)
                        nc.sync.dma_start(
                            dw_t[:, q : q + FB // 4, :], dw_d.ap()[:, q : q + FB // 4, :]
                        )
                    if gi == 1 and fbi in (4, 8):
                        q = FB // 4 * (2 + (fbi - 4) // 4)
                        nc.sync.dma_start(
                            dw_t[:, q : q + FB // 4, :], dw_d.ap()[:, q : q + FB // 4, :]
                        )
                    tt = 0
                    for ti, nt in enumerate(_token_tiles(g)):
                        ps_g = psg.tile([P, 512], f32, name="ps_g")[:, :nt]
                        ps_u = psu.tile([P, 512], f32, name="ps_u")[:, :nt]
                        # First tile of the program: interleave the gate/up
                        # chains so successive h-chunks of x are consumed at
                        # half the rate while the first x chunk streams in.
                        if gi == 0 and fbi == 0 and ti == 0:
                            chains = [
                                (ps, wt_, hc)
                                for hc in range(HC)
                                for ps, wt_ in ((ps_g, gw_t), (ps_u, uw_t))
                            ]
                        else:
                            chains = [(ps_g, gw_t, hc) for hc in range(HC)] + [
                                (ps_u, uw_t, hc) for hc in range(HC)
                            ]
                        for ps, wt_, hc in chains:
                            nc.tensor.matmul(
                                ps,
                                wt_[:, hc, :],
                                x_t[:, hc, t0 + tt : t0 + tt + nt],
                                start=(hc == 0),
                                stop=(hc == HC - 1),
                            )
                        sg = sgpool.tile([P, 512], f32, name="sg")[:, :nt]
                        nc.scalar.activation(sg, ps_g, getattr(AF, act))
                        nc.vector.tensor_mul(hid_t[:, fbi, tt : tt + nt], sg, ps_u)
                        tt += nt

                # ---- Phase B: y[t, :] = wt[t] * (hid[:, t].T @ dw.T) ----
                # Separate accumulation chains per H-half so half 0's
                # scale+store overlaps half 1's matmuls.
                last_group = gi == len(groups) - 1
                for sub in range(g // P):
                    tt128 = t0 // P + sub
                    # On the program's very last sub-tile, use four H-quarter
                    # chains so only a quarter's scale+store remains after
                    # the final matmul (shrinks the end-of-program drain).
                    nslc = 4 if last_group and sub == g // P - 1 else 2
                    wslc = H_ // nslc
                    for hh in range(nslc):
                        ps_y = psy.tile([P, 512], f32, name="ps_y")[:, :wslc]
                        for fbi in range(FBH):
                            nc.tensor.matmul(
                                ps_y,
                                hid_t[:, fbi, sub * P : (sub + 1) * P],
                                dw_t[:, fb_lo + fbi, hh * wslc : (hh + 1) * wslc],
                                start=(fbi == 0),
                                stop=(fbi == FBH - 1),
                            )
                        y_sb = ypool.tile([P, 512], f32, name="y_sb")[:, :wslc]
                        nc.scalar.activation(
                            y_sb, ps_y, AF.Copy, scale=wt_t[:, tt128 : tt128 + 1]
                        )
                        nc.scalar.dma_start(
                            y_d.ap()[tt128][:, hh * wslc : (hh + 1) * wslc], y_sb
                        )
    nc.compile()
    _PROGRAM_CACHE[key] = nc
    return nc


def _routing(hidden_states, router_w):
    """Replicate the reference's routing ops exactly (same jax ops, default
    platform) so top-2 selection matches bit-for-bit."""
    import jax
    import jax.numpy as jnp

    x = jnp.asarray(hidden_states).reshape(-1, H)
    router_logits = x @ jnp.asarray(router_w).T
    routing_weights = jax.nn.softmax(router_logits.astype(jnp.float32), axis=-1)
    top_k_weights, top_k_index = jax.lax.top_k(routing_weights, TOP_K)
    return np.asarray(top_k_index), np.asarray(top_k_weights, dtype=np.float32)


def _pack_gw(w_half):
    """[F2, H] bf16 -> [FBH, P, HC, P] with partition dim = h-within-chunk."""
    return w_half.reshape(FBH, P, HC, P).transpose(0, 3, 2, 1)


def kernel(hidden_states, router_w, gate_w, up_w, down_w):
    import ml_dtypes
    from concourse.bass_utils import run_bass_kernel_spmd

    bf16 = ml_dtypes.bfloat16
    hidden_states = np.asarray(hidden_states, dtype=np.float32)
    router_w = np.asarray(router_w, dtype=np.float32)
    gate_w = np.asarray(gate_w, dtype=np.float32)
    up_w = np.asarray(up_w, dtype=np.float32)
    down_w = np.asarray(down_w, dtype=np.float32)

    tki, tkw = _routing(hidden_states, router_w)
    xf = hidden_states.reshape(T, H).astype(bf16)

    idx_list, w_list = [], []
    for e in range(E):
        sel = tki == e  # [T, 2]
        tok = sel.any(axis=1)
        idx = np.nonzero(tok)[0]
        w = np.where(sel[:, 0], tkw[:, 0], tkw[:, 1])[idx]
        idx_list.append(idx)
        w_list.append(w.astype(np.float32))

    # Pair heaviest with lightest expert; core 2j+h runs F-half h of both
    # experts of pair j.
    order = sorted(range(E), key=lambda e: -len(idx_list[e]))
    heavy, light = order[: E // 2], list(reversed(order[E // 2 :]))
    C1 = int(math.ceil(max(len(idx_list[e]) for e in heavy) / 128.0)) * 128
    C2 = int(math.ceil(max(len(idx_list[e]) for e in light) / 128.0)) * 128
    C1, C2 = max(C1, 256), max(C2, 256)
    C = C1 + C2
    NT128 = C // P

    nc = _build_program(C1, C2)

    gwb = gate_w.astype(bf16)
    uwb = up_w.astype(bf16)
    dwb = down_w.astype(bf16)

    in_maps = []
    pairs = []
    for j in range(E // 2):
        for h in range(2):
            pe, qe = heavy[j], light[j]
            pairs.append((pe, qe))
            fs = slice(h * F2, (h + 1) * F2)
            xg = np.zeros((C, H), bf16)
            wp = np.zeros((C,), np.float32)
            for (e, c0, cap) in ((pe, 0, C1), (qe, C1, C2)):
                ne = len(idx_list[e])
                xg[c0 : c0 + ne] = xf[idx_list[e]]
                wp[c0 : c0 + ne] = w_list[e]
            in_maps.append(
                {
                    "x": np.ascontiguousarray(xg.T.reshape(HC, P, C).transpose(1, 0, 2)),
                    "gw": np.ascontiguousarray(
                        np.concatenate([_pack_gw(gwb[pe][fs]), _pack_gw(gwb[qe][fs])])
                    ),
                    "uw": np.ascontiguousarray(
                        np.concatenate([_pack_gw(uwb[pe][fs]), _pack_gw(uwb[qe][fs])])
                    ),
                    "dw": np.ascontiguousarray(
                        np.concatenate(
                            [
                                dwb[pe].T[fs].reshape(FBH, P, H),
                                dwb[qe].T[fs].reshape(FBH, P, H),
                            ]
                        ).transpose(1, 0, 2)
                    ),
                    "wt": np.ascontiguousarray(wp.reshape(NT128, P)),
                }
            )

    res = run_bass_kernel_spmd(nc, in_maps, core_ids=list(range(N_CORES)))

    out = np.zeros((T, H), np.float32)
    for k in range(N_CORES):
        pe, qe = pairs[k]
        y = res.results[k]["y"].reshape(C, H)
        out[idx_list[pe]] += y[: len(idx_list[pe])]
        out[idx_list[qe]] += y[C1 : C1 + len(idx_list[qe])]
    return out.reshape(B, S, H)


# revision 31
# speedup vs baseline: 1.2600x; 1.0084x over previous
"""Jamba sparse-MoE block on 8 Trainium2 NeuronCores.

Strategy: tensor-parallel ffn (F/8 per core), host dispatch
--------
- Routing (router matmul + softmax + top-2) is computed with jax on the host
  using the exact op sequence of the reference so expert selection matches
  bit-for-bit (one token in the dataset has a top2/top3 probability gap of
  ~5e-7; any rounding difference there would flip its expert assignment).
- Every expert's FFN dim is split 8 ways (F=4096 -> 8x512); core k holds the
  k-th F-slice of ALL experts and processes the whole expert-sorted token
  stream. Per-core work is exactly sum_e ceil(L_e/128)*128 / 8 token-slots —
  the global load-balance floor: no core is pinned by the heaviest expert.
  The 8 partial outputs per token are summed on the host scatter-add.
- All matmul operands are bf16 (same PE rate as float32r on TRN2 — 1 row/cyc
  — but half the DMA bytes and no >=256 free-dim constraint). PSUM fp32.
- Each expert's token range is processed in ~1k-token groups. Per group,
  phase A (hid = silu(x@gw.T) * (x@uw.T)) keeps hid in SBUF as bf16 — no
  DRAM round-trip — and phase B (y = wt * (hid.T @ dw.T)) immediately
  consumes it. x and gate/up weights stream one group ahead; down weights
  load once, early, and stay resident.
- The cost model serializes all DMA on one device, so every input load is
  emitted on the sync queue in consumption-deadline order; y stores use the
  scalar queue.
"""

import math
import numpy as np
from contextlib import ExitStack

B, S, H, F, E, TOP_K = 4, 2048, 1024, 4096, 8, 2
T = B * S
N_CORES = 8
P = 128
HC = H // P  # 8 h-chunks
F8 = F // N_CORES  # 512 ffn rows per core per expert
SFB = F8 // P  # 4 f-blocks per expert segment
FB = E * SFB  # 32 f-blocks held per core
GSZ = 2176  # target tokens per phase-A/phase-B group (one expert segment)


def _token_tiles(g):
    """512-token phase-A tiles covering a group of g tokens (g % 128 == 0)."""
    tiles = [512] * (g // 512)
    if g % 512:
        tiles.append(g % 512)
    return tiles


def _split_groups(Cs):
    """Split a segment of Cs tokens into near-equal 128-multiple groups of at
    most GSZ+128 (one group per expert segment when it fits): fewer, larger
    groups mean fewer phase transitions (each PE idle gap costs ~3us of
    p-state ramp) while per-fb PE work stays far above the per-fb weight DMA
    time so the gate/up stream never starves the PE."""
    nt = Cs // P
    n = max(1, -(-nt // (GSZ // P + 1)))
    out, t0 = [], 0
    for i in range(n):
        take = (nt * (i + 1) // n - nt * i // n) * P
        if take:
            out.append((t0, take))
            t0 += take
    return out


_PROGRAM_CACHE = {}


def _build_program(caps, H_=H, F_=F, act="Silu"):
    """SPMD program: one F/8-slice segment per expert, caps[e] token slots."""
    key = (tuple(caps), H_, F_, act)
    if key in _PROGRAM_CACHE:
        return _PROGRAM_CACHE[key]
    import concourse.bacc as bacc
    import concourse.mybir as mybir
    import concourse.tile as tile

    HC = H_ // P
    f32 = mybir.dt.float32
    bf16 = mybir.dt.bfloat16
    AF = mybir.ActivationFunctionType
    C = sum(caps)
    NT128 = C // P

    # (token_offset, group_len, fb_lo): expert e's F-slice occupies f-blocks
    # 4e..4e+4 and token slots [sum(caps[:e]), sum(caps[:e+1])).
    groups = []
    base = 0
    for e, Ce in enumerate(caps):
        for lt, g in _split_groups(Ce):
            groups.append((base + lt, g, SFB * e))
        base += Ce

    nc = bacc.Bacc("TRN2", target_bir_lowering=False, debug=False, num_devices=N_CORES)

    x_d = nc.dram_tensor("x", [P, HC, C], bf16, kind="ExternalInput")
    gw_d = nc.dram_tensor("gw", [FB, P, HC, P], bf16, kind="ExternalInput")
    uw_d = nc.dram_tensor("uw", [FB, P, HC, P], bf16, kind="ExternalInput")
    dw_d = nc.dram_tensor("dw", [P, FB, H_], bf16, kind="ExternalInput")
    wt_d = nc.dram_tensor("wt", [NT128, P], f32, kind="ExternalInput")
    y_d = nc.dram_tensor("y", [NT128, P, H_], bf16, kind="ExternalOutput")

    hid_max = max(g for _, g, _ in groups)

    with tile.TileContext(nc) as tc:
        with ExitStack() as ctx:
            wtpool = ctx.enter_context(tc.tile_pool(name="wtp", bufs=1))
            xpool = ctx.enter_context(tc.tile_pool(name="xp", bufs=2))
            dwpool = ctx.enter_context(tc.tile_pool(name="dwp", bufs=1))
            gwpool = ctx.enter_context(tc.tile_pool(name="gwp", bufs=4))
            uwpool = ctx.enter_context(tc.tile_pool(name="uwp", bufs=4))
            sgpool = ctx.enter_context(tc.tile_pool(name="sgp", bufs=2))
            hidpool = ctx.enter_context(tc.tile_pool(name="hidp", bufs=1))
            ypool = ctx.enter_context(tc.tile_pool(name="yp", bufs=4))
            psg = ctx.enter_context(tc.tile_pool(name="psg", bufs=2, space="PSUM"))
            psu = ctx.enter_context(tc.tile_pool(name="psu", bufs=2, space="PSUM"))
            psy = ctx.enter_context(tc.tile_pool(name="psy", bufs=4, space="PSUM"))

            dw_t = dwpool.tile([P, FB, H_], bf16)
            wt_t = wtpool.tile([P, NT128], f32)

            # Per-group x tiles, loaded one group ahead. x_tiles[gi] is
            # created during group gi-1's phase A (gi=0 upfront).
            x_tiles = [None] * len(groups)

            def load_x(gi, c_lo, c_hi, hc_step=HC):
                t0, g, _ = groups[gi]
                if x_tiles[gi] is None:
                    x_tiles[gi] = xpool.tile([P, HC, hid_max], bf16, name="x_t")
                xt = x_tiles[gi]
                for c0 in range(c_lo, min(c_hi, g), 512):
                    cn = min(512, g - c0)
                    for hc in range(0, HC, hc_step):
                        nc.sync.dma_start(
                            xt[:, hc : hc + hc_step, c0 : c0 + cn],
                            x_d.ap()[:, hc : hc + hc_step, t0 + c0 : t0 + c0 + cn],
                        )

            for gi, (t0, g, fb_lo) in enumerate(groups):
                # ---- Phase A: hid[f, t] = silu(g) * u, bf16 in SBUF ----
                hid_t = hidpool.tile([P, SFB, hid_max], bf16, name="hid_t")
                first_of_expert = gi == 0 or groups[gi - 1][2] != fb_lo
                for fbi in range(SFB):
                    fb = fb_lo + fbi
                    gw_t = gwpool.tile([P, HC, P], bf16, name="gw_t")
                    nc.sync.dma_start(gw_t[:], gw_d.ap()[fb])
                    if gi == 0 and fbi == 0:
                        load_x(0, 0, 512, hc_step=2)
                    uw_t = uwpool.tile([P, HC, P], bf16, name="uw_t")
                    nc.sync.dma_start(uw_t[:], uw_d.ap()[fb])
                    if gi == 0 and fbi == 0:
                        load_x(0, 512, g, hc_step=4)
                    # next group's tokens stream during this group's phase A
                    if fbi == 1 and gi + 1 < len(groups):
                        load_x(gi + 1, 0, groups[gi + 1][1], hc_step=4 if gi == 0 else HC)
                    # down weights for expert e, during its first group's
                    # phase A (phase B needs them ~2 f-blocks later)
                    if fbi == 2 and first_of_expert:
                        nc.sync.dma_start(
                            dw_t[:, fb_lo : fb_lo + SFB, :],
                            dw_d.ap()[:, fb_lo : fb_lo + SFB, :],
                        )
                    if gi == 0 and fbi == 3:
                        nc.sync.dma_start(wt_t[:], wt_d.ap().rearrange("n p -> p n"))
                    x_t = x_tiles[gi]
                    tt = 0
                    for nt in _token_tiles(g):
                        ps_g = psg.tile([P, 512], f32, name="ps_g")[:, :nt]
                        ps_u = psu.tile([P, 512], f32, name="ps_u")[:, :nt]
                        chains = [(ps_g, gw_t, hc) for hc in range(HC)] + [
                            (ps_u, uw_t, hc) for hc in range(HC)
                        ]
                        for ps, wt_, hc in chains:
                            nc.tensor.matmul(
                                ps,
                                wt_[:, hc, :],
                                x_t[:, hc, tt : tt + nt],
                                start=(hc == 0),
                                stop=(hc == HC - 1),
                            )
                        sg = sgpool.tile([P, 512], f32, name="sg")[:, :nt]
                        nc.scalar.activation(sg, ps_g, getattr(AF, act))
                        nc.vector.tensor_mul(hid_t[:, fbi, tt : tt + nt], sg, ps_u)
                        tt += nt

                # ---- Phase B: y[t, :] = wt[t] * (hid[:, t].T @ dw.T) ----
                # Phase B has only ~1.7us of PE work per 128-token sub-tile,
                # so the scale runs on the (otherwise idle) DVE and the bf16
                # store issues from the SP queue — keeping the Act engine and
                # its HWDGE issue path out of phase B entirely. Separate
                # accumulation chains per H-half so half 0's scale+store
                # overlaps half 1's matmuls.
                last_group = gi == len(groups) - 1
                for sub in range(g // P):
                    tt128 = t0 // P + sub
                    # On the program's very last sub-tile, use four H-quarter
                    # chains so only a quarter's scale+store remains after
                    # the final matmul (shrinks the end-of-program drain).
                    nslc = 4 if last_group and sub == g // P - 1 else 2
                    wslc = H_ // nslc
                    y_sb = ypool.tile([P, H_], bf16, name="y_sb")
                    for hh in range(nslc):
                        ps_y = psy.tile([P, 512], f32, name="ps_y")[:, :wslc]
                        for fbi in range(SFB):
                            nc.tensor.matmul(
                                ps_y,
                                hid_t[:, fbi, sub * P : (sub + 1) * P],
                                dw_t[:, fb_lo + fbi, hh * wslc : (hh + 1) * wslc],
                                start=(fbi == 0),
                                stop=(fbi == SFB - 1),
                            )
                        nc.vector.tensor_scalar_mul(
                            y_sb[:, hh * wslc : (hh + 1) * wslc],
                            ps_y,
                            wt_t[:, tt128 : tt128 + 1],
                        )
                        if nslc == 4:
                            # last sub-tile: store per quarter so only a
                            # quarter's store remains after the final matmul
                            nc.sync.dma_start(
                                y_d.ap()[tt128][:, hh * wslc : (hh + 1) * wslc],
                                y_sb[:, hh * wslc : (hh + 1) * wslc],
                            )
                    if nslc == 2:
                        nc.sync.dma_start(y_d.ap()[tt128], y_sb[:])
    nc.compile()
    _PROGRAM_CACHE[key] = nc
    return nc


def _routing(hidden_states, router_w):
    """Replicate the reference's routing ops exactly (same jax ops, default
    platform) so top-2 selection matches bit-for-bit."""
    import jax
    import jax.numpy as jnp

    x = jnp.asarray(hidden_states).reshape(-1, H)
    router_logits = x @ jnp.asarray(router_w).T
    routing_weights = jax.nn.softmax(router_logits.astype(jnp.float32), axis=-1)
    top_k_weights, top_k_index = jax.lax.top_k(routing_weights, TOP_K)
    return np.asarray(top_k_index), np.asarray(top_k_weights, dtype=np.float32)


def kernel(hidden_states, router_w, gate_w, up_w, down_w):
    import ml_dtypes
    from concourse.bass_utils import run_bass_kernel_spmd

    bf16 = ml_dtypes.bfloat16
    hidden_states = np.asarray(hidden_states, dtype=np.float32)
    router_w = np.asarray(router_w, dtype=np.float32)
    gate_w = np.asarray(gate_w, dtype=np.float32)
    up_w = np.asarray(up_w, dtype=np.float32)
    down_w = np.asarray(down_w, dtype=np.float32)

    tki, tkw = _routing(hidden_states, router_w)
    xf = hidden_states.reshape(T, H).astype(bf16)

    idx_list, w_list = [], []
    for e in range(E):
        sel = tki == e  # [T, 2]
        tok = sel.any(axis=1)
        idx = np.nonzero(tok)[0]
        w = np.where(sel[:, 0], tkw[:, 0], tkw[:, 1])[idx]
        idx_list.append(idx)
        w_list.append(w.astype(np.float32))

    caps = tuple(
        max(256, int(math.ceil(len(idx_list[e]) / 128.0)) * 128) for e in range(E)
    )
    C = sum(caps)
    NT128 = C // P

    nc = _build_program(caps)

    # x / wt: the global expert-sorted padded token stream, same on all cores
    xg = np.zeros((C, H), bf16)
    wp = np.zeros((C,), np.float32)
    base = 0
    for e in range(E):
        ne = len(idx_list[e])
        xg[base : base + ne] = xf[idx_list[e]]
        wp[base : base + ne] = w_list[e]
        base += caps[e]
    x_in = np.ascontiguousarray(xg.T.reshape(HC, P, C).transpose(1, 0, 2))
    wt_in = np.ascontiguousarray(wp.reshape(NT128, P))

    gwb = gate_w.astype(bf16)
    uwb = up_w.astype(bf16)
    dwb = np.ascontiguousarray(down_w.transpose(0, 2, 1)).astype(bf16)  # [E, F, H]

    in_maps = []
    for k in range(N_CORES):
        fs = slice(k * F8, (k + 1) * F8)
        # [E, F8, H] -> blocks of 128 f-rows, partition dim = h-within-chunk
        gslc = gwb[:, fs].reshape(FB, P, HC, P).transpose(0, 3, 2, 1)
        uslc = uwb[:, fs].reshape(FB, P, HC, P).transpose(0, 3, 2, 1)
        dslc = dwb[:, fs].reshape(FB, P, H).transpose(1, 0, 2)
        in_maps.append(
            {
                "x": x_in,
                "gw": np.ascontiguousarray(gslc),
                "uw": np.ascontiguousarray(uslc),
                "dw": np.ascontiguousarray(dslc),
                "wt": wt_in,
            }
        )

    res = run_bass_kernel_spmd(nc, in_maps, core_ids=list(range(N_CORES)))

    ysum = res.results[0]["y"].reshape(C, H).astype(np.float64)
    for k in range(1, N_CORES):
        ysum += res.results[k]["y"].reshape(C, H)
    ysum = ysum.astype(np.float32)

    out = np.zeros((T, H), np.float32)
    base = 0
    for e in range(E):
        ne = len(idx_list[e])
        out[idx_list[e]] += ysum[base : base + ne]
        base += caps[e]
    return out.reshape(B, S, H)


# revision 38
# speedup vs baseline: 1.2790x; 1.0151x over previous
"""Jamba sparse-MoE block on 8 Trainium2 NeuronCores.

Strategy: tensor-parallel ffn (F/8 per core), host dispatch
--------
- Routing (router matmul + softmax + top-2) is computed with jax on the host
  using the exact op sequence of the reference so expert selection matches
  bit-for-bit (one token in the dataset has a top2/top3 probability gap of
  ~5e-7; any rounding difference there would flip its expert assignment).
- Every expert's FFN dim is split 8 ways (F=4096 -> 8x512); core k holds the
  k-th F-slice of ALL experts and processes the whole expert-sorted token
  stream. Per-core work is exactly sum_e ceil(L_e/128)*128 / 8 token-slots —
  the global load-balance floor: no core is pinned by the heaviest expert.
  The 8 partial outputs per token are summed on the host scatter-add.
- All matmul operands are bf16 (same PE rate as float32r on TRN2 — 1 row/cyc
  — but half the DMA bytes and no >=256 free-dim constraint). PSUM fp32.
- Each expert's token range is one phase-A/phase-B group (~2k tokens).
  Phase A (hid = silu(x@gw.T) * (x@uw.T)) keeps hid in SBUF as bf16 — no
  DRAM round-trip — and phase B (y = wt * (hid.T @ dw.T)) immediately
  consumes it. x and gate/up weights stream one group ahead; down weights
  load once, early, and stay resident. Phase B's scale runs on the DVE and
  stores issue from SP, keeping every engine's issue path under the PE rate.
- The cost model serializes all DMA on one device, so every input load is
  emitted on the sync queue in consumption-deadline order; y stores use the
  scalar queue.
"""

import math
import numpy as np
from contextlib import ExitStack

B, S, H, F, E, TOP_K = 4, 2048, 1024, 4096, 8, 2
T = B * S
N_CORES = 8
P = 128
HC = H // P  # 8 h-chunks
F8 = F // N_CORES  # 512 ffn rows per core per expert
SFB = F8 // P  # 4 f-blocks per expert segment
FB = E * SFB  # 32 f-blocks held per core
GSZ = 2176  # target tokens per phase-A/phase-B group (one expert segment)


def _token_tiles(g):
    """512-token phase-A tiles covering a group of g tokens (g % 128 == 0)."""
    tiles = [512] * (g // 512)
    if g % 512:
        tiles.append(g % 512)
    return tiles


def _split_groups(Cs):
    """Split a segment of Cs tokens into near-equal 128-multiple groups of at
    most GSZ+128 (one group per expert segment when it fits): fewer, larger
    groups mean fewer phase transitions (each PE idle gap costs ~3us of
    p-state ramp) while per-fb PE work stays far above the per-fb weight DMA
    time so the gate/up stream never starves the PE."""
    nt = Cs // P
    n = max(1, -(-nt // (GSZ // P + 1)))
    out, t0 = [], 0
    for i in range(n):
        take = (nt * (i + 1) // n - nt * i // n) * P
        if take:
            out.append((t0, take))
            t0 += take
    return out


_PROGRAM_CACHE = {}


def _build_program(caps, loads, H_=H, F_=F, act="Silu"):
    """SPMD program: one F/8-slice segment per expert, caps[e] token slots of
    which only loads[e] are real. Phase A (cost ~ token count) tiles over the
    exact loads; phase B keeps the 128-slot grid (its cost is per H-column,
    independent of token-partition occupancy). Padded slots' hid is never
    written — their phase-B output is garbage scaled by wt=0, and the host
    drops those rows anyway."""
    key = (tuple(caps) + tuple(loads), H_, F_, act)
    if key in _PROGRAM_CACHE:
        return _PROGRAM_CACHE[key]
    import concourse.bacc as bacc
    import concourse.mybir as mybir
    import concourse.tile as tile

    HC = H_ // P
    f32 = mybir.dt.float32
    bf16 = mybir.dt.bfloat16
    AF = mybir.ActivationFunctionType
    C = sum(caps)
    NT128 = C // P

    # (token_offset, group_len, fb_lo, real_len): expert e's F-slice occupies
    # f-blocks 4e..4e+4 and token slots [sum(caps[:e]), sum(caps[:e+1]));
    # only the first loads[e] slots hold real tokens.
    groups = []
    base = 0
    for e, Ce in enumerate(caps):
        for lt, g in _split_groups(Ce):
            lr = max(0, min(g, loads[e] - lt))
            groups.append((base + lt, g, SFB * e, lr))
        base += Ce

    nc = bacc.Bacc("TRN2", target_bir_lowering=False, debug=False, num_devices=N_CORES)

    x_d = nc.dram_tensor("x", [P, HC, C], bf16, kind="ExternalInput")
    gw_d = nc.dram_tensor("gw", [FB, P, HC, P], bf16, kind="ExternalInput")
    uw_d = nc.dram_tensor("uw", [FB, P, HC, P], bf16, kind="ExternalInput")
    dw_d = nc.dram_tensor("dw", [P, FB, H_], bf16, kind="ExternalInput")
    wt_d = nc.dram_tensor("wt", [NT128, P], f32, kind="ExternalInput")
    y_d = nc.dram_tensor("y", [NT128, P, H_], bf16, kind="ExternalOutput")

    hid_max = max(g for _, g, _, _ in groups)

    with tile.TileContext(nc) as tc:
        with ExitStack() as ctx:
            wtpool = ctx.enter_context(tc.tile_pool(name="wtp", bufs=1))
            xpool = ctx.enter_context(tc.tile_pool(name="xp", bufs=2))
            dwpool = ctx.enter_context(tc.tile_pool(name="dwp", bufs=1))
            gwpool = ctx.enter_context(tc.tile_pool(name="gwp", bufs=4))
            uwpool = ctx.enter_context(tc.tile_pool(name="uwp", bufs=4))
            sgpool = ctx.enter_context(tc.tile_pool(name="sgp", bufs=2))
            hidpool = ctx.enter_context(tc.tile_pool(name="hidp", bufs=1))
            ypool = ctx.enter_context(tc.tile_pool(name="yp", bufs=4))
            psg = ctx.enter_context(tc.tile_pool(name="psg", bufs=2, space="PSUM"))
            psu = ctx.enter_context(tc.tile_pool(name="psu", bufs=2, space="PSUM"))
            psy = ctx.enter_context(tc.tile_pool(name="psy", bufs=4, space="PSUM"))

            dw_t = dwpool.tile([P, FB, H_], bf16)
            wt_t = wtpool.tile([P, NT128], f32)

            # Per-group x tiles, loaded one group ahead. x_tiles[gi] is
            # created during group gi-1's phase A (gi=0 upfront).
            x_tiles = [None] * len(groups)

            def load_x(gi, c_lo, c_hi, hc_step=HC):
                # only the real tokens [0, lr) are loaded — padded columns
                # are never read by phase A
                t0, g, _, lr = groups[gi]
                if x_tiles[gi] is None:
                    x_tiles[gi] = xpool.tile([P, HC, hid_max], bf16, name="x_t")
                xt = x_tiles[gi]
                for c0 in range(c_lo, min(c_hi, lr), 512):
                    cn = min(512, lr - c0)
                    for hc in range(0, HC, hc_step):
                        nc.sync.dma_start(
                            xt[:, hc : hc + hc_step, c0 : c0 + cn],
                            x_d.ap()[:, hc : hc + hc_step, t0 + c0 : t0 + c0 + cn],
                        )

            for gi, (t0, g, fb_lo, lr) in enumerate(groups):
                # ---- Phase A: hid[f, t] = silu(g) * u, bf16 in SBUF ----
                hid_t = hidpool.tile([P, SFB, hid_max], bf16, name="hid_t")
                first_of_expert = gi == 0 or groups[gi - 1][2] != fb_lo
                for fbi in range(SFB):
                    fb = fb_lo + fbi
                    gw_t = gwpool.tile([P, HC, P], bf16, name="gw_t")
                    nc.sync.dma_start(gw_t[:], gw_d.ap()[fb])
                    if gi == 0 and fbi == 0:
                        load_x(0, 0, 512, hc_step=2)
                    uw_t = uwpool.tile([P, HC, P], bf16, name="uw_t")
                    nc.sync.dma_start(uw_t[:], uw_d.ap()[fb])
                    if gi == 0 and fbi == 0:
                        load_x(0, 512, lr, hc_step=4)
                    # next group's tokens stream during this group's phase A
                    if fbi == 1 and gi + 1 < len(groups):
                        load_x(gi + 1, 0, groups[gi + 1][3], hc_step=4 if gi == 0 else HC)
                    # down weights for expert e, during its first group's
                    # phase A (phase B needs them ~2 f-blocks later)
                    if fbi == 2 and first_of_expert:
                        nc.sync.dma_start(
                            dw_t[:, fb_lo : fb_lo + SFB, :],
                            dw_d.ap()[:, fb_lo : fb_lo + SFB, :],
                        )
                    if gi == 0 and fbi == 3:
                        nc.sync.dma_start(wt_t[:], wt_d.ap().rearrange("n p -> p n"))
                    x_t = x_tiles[gi]
                    tt = 0
                    for nt in _token_tiles(lr):
                        ps_g = psg.tile([P, 512], f32, name="ps_g")[:, :nt]
                        ps_u = psu.tile([P, 512], f32, name="ps_u")[:, :nt]
                        chains = [(ps_g, gw_t, hc) for hc in range(HC)] + [
                            (ps_u, uw_t, hc) for hc in range(HC)
                        ]
                        for ps, wt_, hc in chains:
                            nc.tensor.matmul(
                                ps,
                                wt_[:, hc, :],
                                x_t[:, hc, tt : tt + nt],
                                start=(hc == 0),
                                stop=(hc == HC - 1),
                            )
                        sg = sgpool.tile([P, 512], f32, name="sg")[:, :nt]
                        nc.scalar.activation(sg, ps_g, getattr(AF, act))
                        nc.vector.tensor_mul(hid_t[:, fbi, tt : tt + nt], sg, ps_u)
                        tt += nt

                # ---- Phase B: y[t, :] = wt[t] * (hid[:, t].T @ dw.T) ----
                # Phase B has only ~1.7us of PE work per 128-token sub-tile,
                # so the scale runs on the (otherwise idle) DVE and the bf16
                # store issues from the SP queue — keeping the Act engine and
                # its HWDGE issue path out of phase B entirely. Separate
                # accumulation chains per H-half so half 0's scale+store
                # overlaps half 1's matmuls.
                last_group = gi == len(groups) - 1
                for sub in range(g // P):
                    tt128 = t0 // P + sub
                    # On the program's very last sub-tile, use four H-quarter
                    # chains so only a quarter's scale+store remains after
                    # the final matmul (shrinks the end-of-program drain).
                    nslc = 4 if last_group and sub == g // P - 1 else 2
                    wslc = H_ // nslc
                    y_sb = ypool.tile([P, H_], bf16, name="y_sb")
                    for hh in range(nslc):
                        ps_y = psy.tile([P, 512], f32, name="ps_y")[:, :wslc]
                        for fbi in range(SFB):
                            nc.tensor.matmul(
                                ps_y,
                                hid_t[:, fbi, sub * P : (sub + 1) * P],
                                dw_t[:, fb_lo + fbi, hh * wslc : (hh + 1) * wslc],
                                start=(fbi == 0),
                                stop=(fbi == SFB - 1),
                            )
                        nc.vector.tensor_scalar_mul(
                            y_sb[:, hh * wslc : (hh + 1) * wslc],
                            ps_y,
                            wt_t[:, tt128 : tt128 + 1],
                        )
                        if nslc == 4:
                            # last sub-tile: store per quarter so only a
                            # quarter's store remains after the final matmul
                            nc.sync.dma_start(
                                y_d.ap()[tt128][:, hh * wslc : (hh + 1) * wslc],
                                y_sb[:, hh * wslc : (hh + 1) * wslc],
                            )
                    if nslc == 2:
                        nc.sync.dma_start(y_d.ap()[tt128], y_sb[:])
    nc.compile()
    _PROGRAM_CACHE[key] = nc
    return nc


def _routing(hidden_states, router_w):
    """Replicate the reference's routing ops exactly (same jax ops, default
    platform) so top-2 selection matches bit-for-bit."""
    import jax
    import jax.numpy as jnp

    x = jnp.asarray(hidden_states).reshape(-1, H)
    router_logits = x @ jnp.asarray(router_w).T
    routing_weights = jax.nn.softmax(router_logits.astype(jnp.float32), axis=-1)
    top_k_weights, top_k_index = jax.lax.top_k(routing_weights, TOP_K)
    return np.asarray(top_k_index), np.asarray(top_k_weights, dtype=np.float32)


def kernel(hidden_states, router_w, gate_w, up_w, down_w):
    import ml_dtypes
    from concourse.bass_utils import run_bass_kernel_spmd

    bf16 = ml_dtypes.bfloat16
    hidden_states = np.asarray(hidden_states, dtype=np.float32)
    router_w = np.asarray(router_w, dtype=np.float32)
    gate_w = np.asarray(gate_w, dtype=np.float32)
    up_w = np.asarray(up_w, dtype=np.float32)
    down_w = np.asarray(down_w, dtype=np.float32)

    tki, tkw = _routing(hidden_states, router_w)
    xf = hidden_states.reshape(T, H).astype(bf16)

    idx_list, w_list = [], []
    for e in range(E):
        sel = tki == e  # [T, 2]
        tok = sel.any(axis=1)
        idx = np.nonzero(tok)[0]
        w = np.where(sel[:, 0], tkw[:, 0], tkw[:, 1])[idx]
        idx_list.append(idx)
        w_list.append(w.astype(np.float32))

    caps = tuple(
        max(256, int(math.ceil(len(idx_list[e]) / 128.0)) * 128) for e in range(E)
    )
    C = sum(caps)
    NT128 = C // P

    nc = _build_program(caps, tuple(len(idx_list[e]) for e in range(E)))

    # x / wt: the global expert-sorted padded token stream, same on all cores
    xg = np.zeros((C, H), bf16)
    wp = np.zeros((C,), np.float32)
    base = 0
    for e in range(E):
        ne = len(idx_list[e])
        xg[base : base + ne] = xf[idx_list[e]]
        wp[base : base + ne] = w_list[e]
        base += caps[e]
    x_in = np.ascontiguousarray(xg.T.reshape(HC, P, C).transpose(1, 0, 2))
    wt_in = np.ascontiguousarray(wp.reshape(NT128, P))

    gwb = gate_w.astype(bf16)
    uwb = up_w.astype(bf16)
    dwb = np.ascontiguousarray(down_w.transpose(0, 2, 1)).astype(bf16)  # [E, F, H]

    in_maps = []
    for k in range(N_CORES):
        fs = slice(k * F8, (k + 1) * F8)
        # [E, F8, H] -> blocks of 128 f-rows, partition dim = h-within-chunk
        gslc = gwb[:, fs].reshape(FB, P, HC, P).transpose(0, 3, 2, 1)
        uslc = uwb[:, fs].reshape(FB, P, HC, P).transpose(0, 3, 2, 1)
        dslc = dwb[:, fs].reshape(FB, P, H).transpose(1, 0, 2)
        in_maps.append(
            {
                "x": x_in,
                "gw": np.ascontiguousarray(gslc),
                "uw": np.ascontiguousarray(uslc),
                "dw": np.ascontiguousarray(dslc),
                "wt": wt_in,
            }
        )

    res = run_bass_kernel_spmd(nc, in_maps, core_ids=list(range(N_CORES)))

    ysum = res.results[0]["y"].reshape(C, H).astype(np.float64)
    for k in range(1, N_CORES):
        ysum += res.results[k]["y"].reshape(C, H)
    ysum = ysum.astype(np.float32)

    out = np.zeros((T, H), np.float32)
    base = 0
    for e in range(E):
        ne = len(idx_list[e])
        out[idx_list[e]] += ysum[base : base + ne]
        base += caps[e]
    return out.reshape(B, S, H)
